# revision 11
# baseline (speedup 1.0000x reference)
"""Distributed Trainium2 kernel for nn_Attn_77970836292156.

Cross-attention block: fused QKV projection + per-head RMSNorm + RoPE +
bf16 SDPA (4096 keys = 2048 self + 2048 cross) + output projection.

Sharding: tensor-parallel on heads. 16 heads / 8 cores = 2 heads per core.
W_qkv / W_ckv column-sharded by head; every core holds full x, y (transposed,
bf16). Attention runs fully local per core in a transposed layout
(head-dims on partitions, positions on the free axis). An AllToAll converts
head-sharding -> sequence-sharding (payload carries unnormalized PV rows
plus the f32 softmax denominators), then each core normalizes and applies
the full W_out to its 256-row slice.

v3 scheduling notes:
- Input x/y DMAs are chunked so the first projection starts ~3us in.
- V/CV projections use 512-wide moving passes into a vT layout, then
  SBUF->SBUF XBAR transpose DMAs produce the natural [keys, dims] layout;
  the softmax-denominator ones-columns live at slots 128/129 of a 130-pitch
  layout and are picked up by two-block stationary access patterns.
- RMSNorm statistics are dense [8, 512] tiles (masked-mean matmuls), so
  Ln/Exp are two tiny ACT ops and the broadcast back is one PE matmul per
  chunk; PSUM evictions run on the otherwise-idle ACT engine.
- The attention exp is split: ACT handles 3/4 of the scores, DVE computes
  the rest via a bit-trick exp2 (int32 ldexp + quadratic mantissa
  correction, max rel err ~1%) so the ACT engine stops being the pipeline
  governor.
"""

import os

import numpy as np
import ml_dtypes

import concourse.bass as bass
import concourse.tile as tile
from concourse import bacc, mybir
from concourse.bass_utils import run_bass_kernel_spmd

BF16 = mybir.dt.bfloat16
F32 = mybir.dt.float32
I32 = mybir.dt.int32

# Problem constants (hardcoded per spec).
N = 2048        # query positions
M = 2048        # cross positions
NK = N + M      # total keys
D = 1024        # model dim
H = 16          # heads
DH = 64         # head dim
HL = 2          # heads per core
DL = HL * DH    # local head dims = 128
F = 1024        # input features
P = 128
NCORES = 8
EPS = 1e-6
ROPE_BASE = 10000.0
SCALE = 0.125   # 1/sqrt(64)

# exp2 bit-trick constants (DVE path): i = int32(EA*s + EB),
# z0 = bitcast_f32(i); t = bitcast_f32((i & MANT) | ONEBITS) in [1,2);
# exp(s*SCALE) ~= (C2*t^2 + C1*t + C0) * z0
EA = 1512775.3951951857        # 2^23 * log2(e) * SCALE
EB = 1065353216.0              # 127 * 2^23
MANT = 0x007FFFFF
ONEBITS = 0x3F800000
C2 = 0.21591707262155144
C1 = -0.6336030333433263
C0 = 1.4081333219835526

NROW = 132  # a2a block rows: 128 oT + 2x2 rows of f32 denominators

USE_DVE_EXP = False   # bisect toggle: DVE bit-trick exp for 256 cols of h1
USE_XBAR_V = False    # bisect toggle: V via vT + XBAR transpose DMAs

LAST_RESULT = None  # test harness reads exec_time_ns from here


def build_nc():
    nc = bacc.Bacc()

    # ---------------- DRAM parameters ----------------
    xT = nc.declare_dram_parameter("xT", [F, N], BF16, isOutput=False)
    yT = nc.declare_dram_parameter("yT", [F, M], BF16, isOutput=False)
    wq = nc.declare_dram_parameter("wq", [F, DL], BF16, isOutput=False)
    wk = nc.declare_dram_parameter("wk", [F, DL], BF16, isOutput=False)
    wv = nc.declare_dram_parameter("wv", [F, DL], BF16, isOutput=False)
    wck = nc.declare_dram_parameter("wck", [F, DL], BF16, isOutput=False)
    wcv = nc.declare_dram_parameter("wcv", [F, DL], BF16, isOutput=False)
    wo = nc.declare_dram_parameter("wo", [D, D], BF16, isOutput=False)
    bo = nc.declare_dram_parameter("bo", [1, D], BF16, isOutput=False)
    cq = nc.declare_dram_parameter("cq", [P, N], BF16, isOutput=False)
    sq = nc.declare_dram_parameter("sq", [P, N], BF16, isOutput=False)
    ckc = nc.declare_dram_parameter("ckc", [P, NK], BF16, isOutput=False)
    cks = nc.declare_dram_parameter("cks", [P, NK], BF16, isOutput=False)
    hssq = nc.declare_dram_parameter("hssq", [P, 32], BF16, isOutput=False)
    hbsel = nc.declare_dram_parameter("hbsel", [8, 512], BF16, isOutput=False)
    hdsel = nc.declare_dram_parameter("hdsel", [16, 8 * P], BF16, isOutput=False)
    out_ext = nc.declare_dram_parameter("out", [N // NCORES, D], F32, isOutput=True)

    # A2A bounce buffers (collectives can't touch I/O tensors).
    a2a_in = nc.dram_tensor("a2a_in", [2, NCORES, NROW, P], BF16)
    a2a_out = nc.dram_tensor("a2a_out", [2, NCORES, NROW, P], BF16)

    with tile.TileContext(nc) as tc, \
            tc.tile_pool(name="singles", bufs=1) as singles:

        # ---------------- static SBUF loads (ordered for early start) ----
        def load_w(param):
            t = singles.tile([P, 8, DL], BF16, tag=param.name + "_sb")
            nc.sync.dma_start(out=t, in_=param.rearrange("(f p) c -> p f c", p=P))
            return t

        wq_sb = load_w(wq)
        wk_sb = load_w(wk)

        xT_sb = singles.tile([P, 8, N], BF16)
        xr = xT.rearrange("(f p) n -> p f n", p=P)
        for c in range(4):
            nc.sync.dma_start(out=xT_sb[:, :, c * 512:(c + 1) * 512],
                              in_=xr[:, :, c * 512:(c + 1) * 512])

        hssq_sb = singles.tile([P, 4, 8], BF16)
        nc.sync.dma_start(out=hssq_sb, in_=hssq[:, :])
        hbsel_sb = singles.tile([8, 4, P], BF16)
        nc.sync.dma_start(out=hbsel_sb, in_=hbsel[:, :])
        cq_sb = singles.tile([P, N], BF16)
        sq_sb = singles.tile([P, N], BF16)
        nc.sync.dma_start(out=cq_sb, in_=cq[:, :])
        nc.sync.dma_start(out=sq_sb, in_=sq[:, :])
        ckc_sb = singles.tile([P, NK], BF16)
        cks_sb = singles.tile([P, NK], BF16)
        nc.sync.dma_start(out=ckc_sb, in_=ckc[:, :])
        nc.sync.dma_start(out=cks_sb, in_=cks[:, :])

        wv_sb = load_w(wv)
        yT_sb = singles.tile([P, 8, M], BF16)
        yr = yT.rearrange("(f p) n -> p f n", p=P)
        for c in range(4):
            nc.sync.dma_start(out=yT_sb[:, :, c * 512:(c + 1) * 512],
                              in_=yr[:, :, c * 512:(c + 1) * 512])
        wck_sb = load_w(wck)
        wcv_sb = load_w(wcv)

        wo_sb = singles.tile([P, 8, D], BF16)
        nc.sync.dma_start(out=wo_sb, in_=wo.rearrange("(f p) c -> p f c", p=P))
        bo_sb = singles.tile([1, D], BF16)
        nc.sync.dma_start(out=bo_sb, in_=bo[0:1, :])
        hdsel_sb = singles.tile([16, 8, P], BF16)
        nc.sync.dma_start(out=hdsel_sb, in_=hdsel[:, :])

        ones1 = singles.tile([1, P], BF16)
        nc.vector.memset(ones1, 1.0)
        eps8 = singles.tile([8, 1], F32)
        nc.vector.memset(eps8, EPS)

        # Normed/roped activations in transposed layout.
        qTn = singles.tile([P, N], BF16)
        kTn = singles.tile([P, NK], BF16)
        # V natural layout [keys, 32 chunks, 130]: [h0 64 | 1 | h1 64 | 1]
        # (ones columns at 64/129 feed the softmax-denominator row).
        v_nat = singles.tile([P, 32, 130], BF16)
        nc.gpsimd.memset(v_nat, 1.0)
        # Attention output (unnormalized), transposed layout.
        oT = singles.tile([P, N], BF16)

        def v_stat(t, h):
            """Stationary AP [128, 65] for PV: head h dims + ones column."""
            return v_nat[:, t, h * 65:(h + 1) * 65]

        # ---------------- phase 1: projections + RMSNorm + RoPE ----------
        with tc.tile_pool(name="proj_ps", bufs=2, space="PSUM") as proj_ps, \
                tc.tile_pool(name="ssq_ps", bufs=2, space="PSUM") as ssq_ps, \
                tc.tile_pool(name="rsb_ps", bufs=2, space="PSUM") as rsb_ps, \
                tc.tile_pool(name="t1p", bufs=2) as t1p, \
                tc.tile_pool(name="t1sp", bufs=2) as t1sp, \
                tc.tile_pool(name="sqp", bufs=5) as sqp, \
                tc.tile_pool(name="rsp", bufs=2) as rsp, \
                tc.tile_pool(name="rsbep", bufs=2) as rsbep, \
                tc.tile_pool(name="mrp", bufs=3) as mrp, \
                tc.tile_pool(name="t1rp", bufs=3) as t1rp, \
                tc.tile_pool(name="vtp", bufs=2) as vtp:

            def proj_part(w_sb, src_sb):
                """Project 2048 positions (transposed layout); compute
                per-(chunk, head) inverse-rms rows [8, 512]."""
                t1 = t1p.tile([P, N], BF16, tag="t1")
                qsqs = []
                for c in range(4):
                    cs = slice(c * 512, (c + 1) * 512)
                    ps = proj_ps.tile([P, 512], F32)
                    for f in range(8):
                        nc.tensor.matmul(ps, w_sb[:, f, :],
                                         src_sb[:, f, cs],
                                         start=(f == 0), stop=(f == 7))
                    nc.scalar.activation(
                        out=t1[:, cs], in_=ps,
                        func=mybir.ActivationFunctionType.Copy)
                    qsq = sqp.tile([P, 512], BF16, tag="qsq", name=f"qsq{c}")
                    nc.vector.tensor_mul(qsq, t1[:, cs], t1[:, cs])
                    qsqs.append(qsq)
                ssq = ssq_ps.tile([8, 512], F32, tag="ssq")
                for c in range(4):
                    nc.tensor.matmul(ssq, hssq_sb[:, c, :], qsqs[c],
                                     start=(c == 0), stop=(c == 3))
                lns = rsp.tile([8, 512], F32, tag="lns")
                nc.scalar.activation(out=lns, in_=ssq,
                                     func=mybir.ActivationFunctionType.Ln,
                                     bias=eps8)
                rs = rsp.tile([8, 512], BF16, tag="rs")
                nc.scalar.activation(out=rs, in_=lns,
                                     func=mybir.ActivationFunctionType.Exp,
                                     scale=-0.5)
                return t1, rs

            def rope_part(t1, rs, dst, dst_off, c_tab, s_tab, tab_off):
                """Apply inverse-rms scale + rotate-half RoPE, write dst."""
                t1s = t1sp.tile([P, N], BF16, tag="t1s")
                for c in range(4):
                    cs = slice(c * 512, (c + 1) * 512)
                    rsb = rsb_ps.tile([P, 512], F32)
                    nc.tensor.matmul(rsb, hbsel_sb[:, c, :], rs,
                                     start=True, stop=True)
                    rsbe = rsbep.tile([P, 512], BF16, tag="rsbe")
                    nc.scalar.activation(
                        out=rsbe, in_=rsb,
                        func=mybir.ActivationFunctionType.Copy)
                    nc.vector.tensor_mul(t1s[:, cs], t1[:, cs], rsbe)
                    # rotate-half across partitions via SBUF->SBUF DMA
                    t1r = t1rp.tile([P, 512], BF16, tag="t1r")
                    for h in range(HL):
                        b = h * DH
                        nc.sync.dma_start(out=t1r[b:b + 32, :],
                                          in_=t1s[b + 32:b + 64, cs])
                        nc.sync.dma_start(out=t1r[b + 32:b + 64, :],
                                          in_=t1s[b:b + 32, cs])
                    tab = slice(tab_off + c * 512, tab_off + (c + 1) * 512)
                    m1 = mrp.tile([P, 512], BF16, tag="m1")
                    nc.vector.tensor_mul(m1, t1s[:, cs], c_tab[:, tab])
                    r1 = mrp.tile([P, 512], BF16, tag="r1")
                    nc.vector.tensor_mul(r1, t1r, s_tab[:, tab])
                    sl = slice(dst_off + c * 512, dst_off + (c + 1) * 512)
                    nc.vector.tensor_add(dst[:, sl], m1, r1)

            def v_blockc(w_sb, src_sb, tbase, c):
                """V/CV projection chunk: vT via 512-wide passes, then XBAR
                transposes into the natural layout."""
                if not USE_XBAR_V:
                    for u in range(4):
                        t = tbase + 4 * c + u
                        tt = 4 * c + u
                        ps = proj_ps.tile([P, DL], F32)
                        for f in range(8):
                            nc.tensor.matmul(
                                ps, src_sb[:, f, tt * P:(tt + 1) * P],
                                w_sb[:, f, :], start=(f == 0), stop=(f == 7))
                        nc.scalar.activation(
                            out=v_nat[:, t, 0:64], in_=ps[:, 0:64],
                            func=mybir.ActivationFunctionType.Copy)
                        nc.scalar.activation(
                            out=v_nat[:, t, 65:129], in_=ps[:, 64:128],
                            func=mybir.ActivationFunctionType.Copy)
                    return
                ps = proj_ps.tile([P, 512], F32)
                for f in range(8):
                    nc.tensor.matmul(ps, w_sb[:, f, :],
                                     src_sb[:, f, c * 512:(c + 1) * 512],
                                     start=(f == 0), stop=(f == 7))
                vt = vtp.tile([P, 512], BF16, tag="vt")
                nc.scalar.activation(
                    out=vt, in_=ps, func=mybir.ActivationFunctionType.Copy)
                for u in range(4):
                    t = tbase + 4 * c + u
                    usl = slice(u * P, (u + 1) * P)
                    nc.sync.dma_start(out=v_nat[:, t, 0:64],
                                      in_=vt[0:64, usl], transpose=True)
                    nc.sync.dma_start(out=v_nat[:, t, 65:129],
                                      in_=vt[64:128, usl], transpose=True)

            t1q, rsq = proj_part(wq_sb, xT_sb)
            t1k, rsk = proj_part(wk_sb, xT_sb)
            rope_part(t1q, rsq, qTn, 0, cq_sb, sq_sb, 0)
            for c in range(4):
                v_blockc(wv_sb, xT_sb, 0, c)
            rope_part(t1k, rsk, kTn, 0, ckc_sb, cks_sb, 0)
            t1c, rsc_ = proj_part(wck_sb, yT_sb)
            for c in range(4):
                v_blockc(wcv_sb, yT_sb, 16, c)
            rope_part(t1c, rsc_, kTn, N, ckc_sb, cks_sb, N)

        # ---------------- phase 2: attention ----------------
        with tc.tile_pool(name="st_ps", bufs=2, space="PSUM") as st_ps, \
                tc.tile_pool(name="pv_ps", bufs=1, space="PSUM") as pv_ps, \
                tc.tile_pool(name="esp", bufs=2) as esp, \
                tc.tile_pool(name="esp2", bufs=2) as esp2, \
                tc.tile_pool(name="esbp", bufs=2) as esbp, \
                tc.tile_pool(name="yip", bufs=2) as yip, \
                tc.tile_pool(name="mip", bufs=2) as mip, \
                tc.tile_pool(name="uup", bufs=2) as uup, \
                tc.tile_pool(name="vvp", bufs=2) as vvp, \
                tc.tile_pool(name="ddp", bufs=2) as ddp:
            for qh in range(2):
                qsl = slice(qh * 1024, (qh + 1) * 1024)
                pv = [pv_ps.tile([65, 1024], F32, name=f"pv{h}", tag=f"pv{h}")
                      for h in range(HL)]
                for kc in range(NK // P):
                    # --- head 0: full ACT exp ---
                    st0 = st_ps.tile([P, 1024], F32, name="st0", tag="st")
                    for c in range(2):
                        nc.tensor.matmul(
                            st0[:, c * 512:(c + 1) * 512],
                            kTn[0:DH, kc * P:(kc + 1) * P],
                            qTn[0:DH, qh * 1024 + c * 512:
                                qh * 1024 + (c + 1) * 512],
                            start=True, stop=True)
                    e0 = esp.tile([P, 1024], BF16, name="es0", tag="es0")
                    nc.scalar.activation(out=e0, in_=st0,
                                         func=mybir.ActivationFunctionType.Exp,
                                         scale=SCALE)
                    # --- head 1: ACT exp on 768 cols, DVE exp2 on 256 ---
                    st1 = st_ps.tile([P, 1024], F32, name="st1", tag="st")
                    for c in range(2):
                        nc.tensor.matmul(
                            st1[:, c * 512:(c + 1) * 512],
                            kTn[DH:DL, kc * P:(kc + 1) * P],
                            qTn[DH:DL, qh * 1024 + c * 512:
                                qh * 1024 + (c + 1) * 512],
                            start=True, stop=True)
                    nact1 = 768 if USE_DVE_EXP else 1024
                    e1a = esp2.tile([P, nact1], BF16, name="es1a", tag="es1a")
                    nc.scalar.activation(out=e1a, in_=st1[:, 0:nact1],
                                         func=mybir.ActivationFunctionType.Exp,
                                         scale=SCALE)
                    if USE_DVE_EXP:
                        yi = yip.tile([P, 256], I32, tag="yi")
                        nc.vector.tensor_scalar(yi, st1[:, 768:1024], EA, EB,
                                                mybir.AluOpType.mult,
                                                mybir.AluOpType.add)
                        mi = mip.tile([P, 256], I32, tag="mi")
                        nc.vector.tensor_scalar(mi, yi, MANT, ONEBITS,
                                                mybir.AluOpType.bitwise_and,
                                                mybir.AluOpType.bitwise_or)
                        uu = uup.tile([P, 256], F32, tag="uu")
                        nc.vector.tensor_scalar(uu, mi.bitcast(F32), C2, C1,
                                                mybir.AluOpType.mult,
                                                mybir.AluOpType.add)
                        vv = vvp.tile([P, 256], F32, tag="vv")
                        nc.vector.tensor_mul(vv, uu, mi.bitcast(F32))
                        e1b = esbp.tile([P, 256], BF16, tag="e1b")
                        nc.vector.scalar_tensor_tensor(
                            e1b, vv, C0, yi.bitcast(F32),
                            mybir.AluOpType.add, mybir.AluOpType.mult)
                    # --- PV accumulation ---
                    kc_first, kc_last = kc == 0, kc == NK // P - 1
                    for c in range(2):
                        nc.tensor.matmul(
                            pv[0][:, c * 512:(c + 1) * 512],
                            v_stat(kc, 0), e0[:, c * 512:(c + 1) * 512],
                            start=kc_first, stop=kc_last)
                    nc.tensor.matmul(pv[1][:, 0:512], v_stat(kc, 1),
                                     e1a[:, 0:512],
                                     start=kc_first, stop=kc_last)
                    if USE_DVE_EXP:
                        nc.tensor.matmul(pv[1][:, 512:768], v_stat(kc, 1),
                                         e1a[:, 512:768],
                                         start=kc_first, stop=kc_last)
                        nc.tensor.matmul(pv[1][:, 768:1024], v_stat(kc, 1),
                                         e1b, start=kc_first, stop=kc_last)
                    else:
                        nc.tensor.matmul(pv[1][:, 512:1024], v_stat(kc, 1),
                                         e1a[:, 512:1024],
                                         start=kc_first, stop=kc_last)
                # Evict unnormalized oT + denominators, ship via A2A.
                dds = []
                for h in range(HL):
                    nc.vector.tensor_copy(oT[h * DH:(h + 1) * DH, qsl],
                                          pv[h][0:64, :])
                    dd = ddp.tile([1, 1024], F32, tag="dd", name=f"dd{h}")
                    nc.scalar.activation(
                        out=dd, in_=pv[h][64:65, :],
                        func=mybir.ActivationFunctionType.Copy)
                    dds.append(dd)
                for j in range(NCORES):
                    nc.sync.dma_start(
                        out=a2a_in[qh, j, 0:128, :],
                        in_=oT[:, qh * 1024 + j * P: qh * 1024 + (j + 1) * P])
                    for h in range(HL):
                        nc.sync.dma_start(
                            out=a2a_in[qh, j, 128 + 2 * h:130 + 2 * h, :],
                            in_=dds[h][0:1, j * P:(j + 1) * P].bitcast(BF16))
                nc.gpsimd.collective_compute(
                    "AllToAll", mybir.AluOpType.bypass,
                    replica_groups=[list(range(NCORES))],
                    ins=[a2a_in[qh]],
                    outs=[a2a_out[qh]],
                )

        # ---------------- phase 3: normalize + output projection ---------
        with tc.tile_pool(name="p3", bufs=1) as p3, \
                tc.tile_pool(name="djp", bufs=2) as djp, \
                tc.tile_pool(name="rdb_ps", bufs=2, space="PSUM") as rdb_ps, \
                tc.tile_pool(name="z_ps", bufs=2, space="PSUM") as z_ps, \
                tc.tile_pool(name="zout", bufs=2) as zout:
            for qh in range(2):
                of_sb = p3.tile([P, NCORES, P], BF16, name="of_sb",
                                tag=f"of{qh}")
                dj = djp.tile([16, P], F32, tag="dj")
                for i in range(NCORES):
                    nc.sync.dma_start(out=of_sb[:, i, :],
                                      in_=a2a_out[qh, i, 0:128, :])
                    nc.sync.dma_start(out=dj[2 * i:2 * i + 2, :],
                                      in_=a2a_out[qh, i, 128:132, :].bitcast(F32))
                rdj = djp.tile([16, P], F32, tag="rdj")
                nc.vector.reciprocal(rdj, dj)
                rdjb = djp.tile([16, P], BF16, tag="rdjb")
                nc.vector.tensor_copy(rdjb, rdj)
                ofn_sb = p3.tile([P, NCORES, P], BF16, name="ofn_sb",
                                 tag=f"ofn{qh}")
                for i in range(NCORES):
                    rdb = rdb_ps.tile([P, P], F32)
                    nc.tensor.matmul(rdb, hdsel_sb[:, i, :], rdjb,
                                     start=True, stop=True)
                    nc.vector.tensor_mul(ofn_sb[:, i, :], of_sb[:, i, :], rdb)
                for nn in range(2):  # 2 output col chunks of 512
                    zp = z_ps.tile([P, 512], F32)
                    for i in range(NCORES):
                        nc.tensor.matmul(zp, ofn_sb[:, i, :],
                                         wo_sb[:, i, nn * 512:(nn + 1) * 512],
                                         start=(i == 0), stop=False)
                    nc.tensor.matmul(zp, ones1,
                                     bo_sb[:, nn * 512:(nn + 1) * 512],
                                     start=False, stop=True)
                    zs = zout.tile([P, 512], F32)
                    nc.scalar.activation(
                        out=zs, in_=zp,
                        func=mybir.ActivationFunctionType.Copy)
                    nc.sync.dma_start(out=out_ext[qh * P:(qh + 1) * P,
                                                  nn * 512:(nn + 1) * 512],
                                      in_=zs)
    return nc


def _bf16(a):
    return np.ascontiguousarray(a).astype(ml_dtypes.bfloat16)


def _rope_tables(npos, pos0, g_first, g_second, n_first):
    """Tables [128, npos] for transposed-layout rope with g folded in.

    Row j (within a head, duplicated for 2 local heads):
      out[j] = t[j]*C[j] + t[sigma(j)]*S[j]
      j <  32: C[j]=g[j]*cos[n,j],     S[j]=-g[j+32]*sin[n,j]
      j >= 32: C[j]=g[j]*cos[n,j-32],  S[j]=+g[j-32]*sin[n,j-32]
    g switches from g_first to g_second at position n_first.
    """
    inv = 1.0 / (ROPE_BASE ** (np.arange(0, DH, 2, dtype=np.float64) / DH))
    pos = np.arange(pos0, pos0 + npos, dtype=np.float64)
    ang = pos[:, None] * inv[None, :]          # [npos, 32]
    cos = np.cos(ang).T                         # [32, npos]
    sin = np.sin(ang).T
    C = np.zeros((DH, npos), np.float64)
    S = np.zeros((DH, npos), np.float64)
    g = np.zeros((DH, npos), np.float64)
    g[:, :n_first] = np.asarray(g_first, np.float64)[:, None]
    if n_first < npos:
        g[:, n_first:] = np.asarray(g_second, np.float64)[:, None]
    C[:32] = cos
    C[32:] = cos
    C *= g
    S[:32] = -sin
    S[32:] = sin
    Srot = np.concatenate([g[32:], g[:32]], axis=0)  # g[sigma(j)]
    S *= Srot
    C2_ = np.concatenate([C, C], axis=0)  # duplicate for 2 local heads
    S2_ = np.concatenate([S, S], axis=0)
    return _bf16(C2_), _bf16(S2_)


_NC_CACHE = None


def kernel(x, y, W_qkv, W_ckv, W_out, b_out, g_q, g_k, g_ck, n_heads):
    global LAST_RESULT, _NC_CACHE
    x = np.asarray(x, np.float32)
    y = np.asarray(y, np.float32)
    W_qkv = np.asarray(W_qkv, np.float32)
    W_ckv = np.asarray(W_ckv, np.float32)
    W_out = np.asarray(W_out, np.float32)
    b_out = np.asarray(b_out, np.float32)

    xT = _bf16(x[0].T)                       # [1024, 2048]
    yT = _bf16(y[0].T)
    Wq, Wk, Wv = (W_qkv[:, i * D:(i + 1) * D] for i in range(3))
    Wck, Wcv = (W_ckv[:, i * D:(i + 1) * D] for i in range(2))
    woh = _bf16(W_out)
    boh = _bf16(b_out[None, :])

    cqh, sqh = _rope_tables(N, 0, g_q, g_q, N)
    ckch, cksh = _rope_tables(NK, 0, g_k, g_ck, N)

    hssq = np.zeros((P, 4, 8), np.float32)
    hbsel = np.zeros((8, 4, P), np.float32)
    for c in range(4):
        for h in range(HL):
            hssq[h * DH:(h + 1) * DH, c, 2 * c + h] = 1.0 / DH
            hbsel[2 * c + h, c, h * DH:(h + 1) * DH] = 1.0
    hdsel = np.zeros((16, 8, P), np.float32)
    for i in range(NCORES):
        for h in range(HL):
            hdsel[2 * i + h, i, h * DH:(h + 1) * DH] = 1.0

    in_maps = []
    for c in range(NCORES):
        sl = slice(c * DL, (c + 1) * DL)
        in_maps.append({
            "xT": xT, "yT": yT,
            "wq": _bf16(Wq[:, sl]), "wk": _bf16(Wk[:, sl]),
            "wv": _bf16(Wv[:, sl]), "wck": _bf16(Wck[:, sl]),
            "wcv": _bf16(Wcv[:, sl]),
            "wo": woh, "bo": boh,
            "cq": cqh, "sq": sqh, "ckc": ckch, "cks": cksh,
            "hssq": _bf16(hssq.reshape(P, 32)),
            "hbsel": _bf16(hbsel.reshape(8, 512)),
            "hdsel": _bf16(hdsel.reshape(16, 8 * P)),
        })

    if _NC_CACHE is None:
        _NC_CACHE = build_nc()
        if not _NC_CACHE.is_finalized():
            _NC_CACHE.finalize()
    nc = _NC_CACHE

    res = run_bass_kernel_spmd(
        nc, in_maps, core_ids=list(range(NCORES)),
        trace=bool(os.environ.get("BASS_TRACE")),
    )
    LAST_RESULT = res
    out = np.empty((N, D), np.float32)
    for c in range(NCORES):
        o = np.asarray(res.results[c]["out"], np.float32)
        out[c * P:(c + 1) * P] = o[0:P]
        out[N // 2 + c * P:N // 2 + (c + 1) * P] = o[P:2 * P]
    return out[None, :, :]


# revision 15
# speedup vs baseline: 1.0289x; 1.0289x over previous
"""Distributed Trainium2 kernel for nn_Attn_77970836292156.

Cross-attention block: fused QKV projection + per-head RMSNorm + RoPE +
bf16 SDPA (4096 keys = 2048 self + 2048 cross) + output projection.

Sharding: tensor-parallel on heads. 16 heads / 8 cores = 2 heads per core.
W_qkv / W_ckv column-sharded by head; every core holds full x, y (transposed,
bf16). Attention runs fully local per core in a transposed layout
(head-dims on partitions, positions on the free axis). An AllToAll converts
head-sharding -> sequence-sharding (payload carries unnormalized PV rows
plus the f32 softmax denominators), then each core normalizes and applies
the full W_out to its 256-row slice.

v3 scheduling notes:
- Input x/y DMAs are chunked so the first projection starts ~3us in.
- V/CV projections use 512-wide moving passes into a vT layout, then
  SBUF->SBUF XBAR transpose DMAs produce the natural [keys, dims] layout;
  the softmax-denominator ones-columns live at slots 128/129 of a 130-pitch
  layout and are picked up by two-block stationary access patterns.
- RMSNorm statistics are dense [8, 512] tiles (masked-mean matmuls), so
  Ln/Exp are two tiny ACT ops and the broadcast back is one PE matmul per
  chunk; PSUM evictions run on the otherwise-idle ACT engine.
- The attention exp is split: ACT handles 3/4 of the scores, DVE computes
  the rest via a bit-trick exp2 (int32 ldexp + quadratic mantissa
  correction, max rel err ~1%) so the ACT engine stops being the pipeline
  governor.
"""

import os

import numpy as np
import ml_dtypes

import concourse.bass as bass
import concourse.tile as tile
from concourse import bacc, mybir
from concourse.bass_utils import run_bass_kernel_spmd

BF16 = mybir.dt.bfloat16
F32 = mybir.dt.float32
I32 = mybir.dt.int32

# Problem constants (hardcoded per spec).
N = 2048        # query positions
M = 2048        # cross positions
NK = N + M      # total keys
D = 1024        # model dim
H = 16          # heads
DH = 64         # head dim
HL = 2          # heads per core
DL = HL * DH    # local head dims = 128
F = 1024        # input features
P = 128
NCORES = 8
EPS = 1e-6
ROPE_BASE = 10000.0
SCALE = 0.125   # 1/sqrt(64)

# exp2 bit-trick constants (DVE path): i = int32(EA*s + EB),
# z0 = bitcast_f32(i); t = bitcast_f32((i & MANT) | ONEBITS) in [1,2);
# exp(s*SCALE) ~= (C2*t^2 + C1*t + C0) * z0
EA = 1512775.3951951857        # 2^23 * log2(e) * SCALE
EB = 1065353216.0              # 127 * 2^23
MANT = 0x007FFFFF
ONEBITS = 0x3F800000
C2 = 0.21591707262155144
C1 = -0.6336030333433263
C0 = 1.4081333219835526

NROW = 132  # a2a block rows: 128 oT + 2x2 rows of f32 denominators

USE_DVE_EXP = True    # bisect toggle: DVE bit-trick exp for 256 cols of h1
USE_XBAR_V = False    # bisect toggle: V via vT + XBAR transpose DMAs

LAST_RESULT = None  # test harness reads exec_time_ns from here


def build_nc():
    nc = bacc.Bacc()

    # ---------------- DRAM parameters ----------------
    xT = nc.declare_dram_parameter("xT", [F, N], BF16, isOutput=False)
    yT = nc.declare_dram_parameter("yT", [F, M], BF16, isOutput=False)
    wq = nc.declare_dram_parameter("wq", [F, DL], BF16, isOutput=False)
    wk = nc.declare_dram_parameter("wk", [F, DL], BF16, isOutput=False)
    wv = nc.declare_dram_parameter("wv", [F, DL], BF16, isOutput=False)
    wck = nc.declare_dram_parameter("wck", [F, DL], BF16, isOutput=False)
    wcv = nc.declare_dram_parameter("wcv", [F, DL], BF16, isOutput=False)
    wo = nc.declare_dram_parameter("wo", [D, D], BF16, isOutput=False)
    bo = nc.declare_dram_parameter("bo", [1, D], BF16, isOutput=False)
    cq = nc.declare_dram_parameter("cq", [P, N], BF16, isOutput=False)
    sq = nc.declare_dram_parameter("sq", [P, N], BF16, isOutput=False)
    ckc = nc.declare_dram_parameter("ckc", [P, NK], BF16, isOutput=False)
    cks = nc.declare_dram_parameter("cks", [P, NK], BF16, isOutput=False)
    hssq = nc.declare_dram_parameter("hssq", [P, 32], BF16, isOutput=False)
    hbsel = nc.declare_dram_parameter("hbsel", [8, 512], BF16, isOutput=False)
    hdsel = nc.declare_dram_parameter("hdsel", [16, 8 * P], BF16, isOutput=False)
    out_ext = nc.declare_dram_parameter("out", [N // NCORES, D], F32, isOutput=True)

    # A2A bounce buffers (collectives can't touch I/O tensors).
    a2a_in = nc.dram_tensor("a2a_in", [2, NCORES, NROW, P], BF16)
    a2a_out = nc.dram_tensor("a2a_out", [2, NCORES, NROW, P], BF16)

    with tile.TileContext(nc) as tc, \
            tc.tile_pool(name="singles", bufs=1) as singles:

        # ---------------- static SBUF loads (ordered for early start) ----
        def load_w(param):
            t = singles.tile([P, 8, DL], BF16, tag=param.name + "_sb")
            nc.sync.dma_start(out=t, in_=param.rearrange("(f p) c -> p f c", p=P))
            return t

        wq_sb = load_w(wq)
        wk_sb = load_w(wk)

        xT_sb = singles.tile([P, 8, N], BF16)
        xr = xT.rearrange("(f p) n -> p f n", p=P)
        for c in range(4):
            nc.sync.dma_start(out=xT_sb[:, :, c * 512:(c + 1) * 512],
                              in_=xr[:, :, c * 512:(c + 1) * 512])

        hssq_sb = singles.tile([P, 4, 8], BF16)
        nc.sync.dma_start(out=hssq_sb, in_=hssq[:, :])
        hbsel_sb = singles.tile([8, 4, P], BF16)
        nc.sync.dma_start(out=hbsel_sb, in_=hbsel[:, :])
        cq_sb = singles.tile([P, N], BF16)
        sq_sb = singles.tile([P, N], BF16)
        nc.sync.dma_start(out=cq_sb, in_=cq[:, :])
        nc.sync.dma_start(out=sq_sb, in_=sq[:, :])
        ckc_sb = singles.tile([P, NK], BF16)
        cks_sb = singles.tile([P, NK], BF16)
        nc.sync.dma_start(out=ckc_sb, in_=ckc[:, :])
        nc.sync.dma_start(out=cks_sb, in_=cks[:, :])

        wv_sb = load_w(wv)
        yT_sb = singles.tile([P, 8, M], BF16)
        yr = yT.rearrange("(f p) n -> p f n", p=P)
        for c in range(4):
            nc.sync.dma_start(out=yT_sb[:, :, c * 512:(c + 1) * 512],
                              in_=yr[:, :, c * 512:(c + 1) * 512])
        wck_sb = load_w(wck)
        wcv_sb = load_w(wcv)

        wo_sb = singles.tile([P, 8, D], BF16)
        nc.sync.dma_start(out=wo_sb, in_=wo.rearrange("(f p) c -> p f c", p=P))
        bo_sb = singles.tile([1, D], BF16)
        nc.sync.dma_start(out=bo_sb, in_=bo[0:1, :])
        hdsel_sb = singles.tile([16, 8, P], BF16)
        nc.sync.dma_start(out=hdsel_sb, in_=hdsel[:, :])

        ones1 = singles.tile([1, P], BF16)
        nc.vector.memset(ones1, 1.0)
        eps8 = singles.tile([8, 1], F32)
        nc.vector.memset(eps8, EPS)

        # Normed/roped activations in transposed layout.
        qTn = singles.tile([P, N], BF16)
        kTn = singles.tile([P, NK], BF16)
        # V natural layout [keys, 32 chunks, 130]: [h0 64 | 1 | h1 64 | 1]
        # (ones columns at 64/129 feed the softmax-denominator row).
        v_nat = singles.tile([P, 32, 130], BF16)
        nc.gpsimd.memset(v_nat, 1.0)
        # Attention output (unnormalized), transposed layout.
        oT = singles.tile([P, N], BF16)

        def v_stat(t, h):
            """Stationary AP [128, 65] for PV: head h dims + ones column."""
            return v_nat[:, t, h * 65:(h + 1) * 65]

        # ---------------- phase 1: projections + RMSNorm + RoPE ----------
        with tc.tile_pool(name="proj_ps", bufs=2, space="PSUM") as proj_ps, \
                tc.tile_pool(name="ssq_ps", bufs=2, space="PSUM") as ssq_ps, \
                tc.tile_pool(name="rsb_ps", bufs=2, space="PSUM") as rsb_ps, \
                tc.tile_pool(name="t1p", bufs=2) as t1p, \
                tc.tile_pool(name="t1sp", bufs=2) as t1sp, \
                tc.tile_pool(name="sqp", bufs=5) as sqp, \
                tc.tile_pool(name="rsp", bufs=2) as rsp, \
                tc.tile_pool(name="rsbep", bufs=2) as rsbep, \
                tc.tile_pool(name="mrp", bufs=3) as mrp, \
                tc.tile_pool(name="t1rp", bufs=3) as t1rp, \
                tc.tile_pool(name="vtp", bufs=2) as vtp:

            def proj_part(w_sb, src_sb):
                """Project 2048 positions (transposed layout); compute
                per-(chunk, head) inverse-rms rows [8, 512]."""
                t1 = t1p.tile([P, N], BF16, tag="t1")
                qsqs = []
                for c in range(4):
                    cs = slice(c * 512, (c + 1) * 512)
                    ps = proj_ps.tile([P, 512], F32)
                    for f in range(8):
                        nc.tensor.matmul(ps, w_sb[:, f, :],
                                         src_sb[:, f, cs],
                                         start=(f == 0), stop=(f == 7))
                    nc.scalar.activation(
                        out=t1[:, cs], in_=ps,
                        func=mybir.ActivationFunctionType.Copy)
                    qsq = sqp.tile([P, 512], BF16, tag="qsq", name=f"qsq{c}")
                    nc.vector.tensor_mul(qsq, t1[:, cs], t1[:, cs])
                    qsqs.append(qsq)
                ssq = ssq_ps.tile([8, 512], F32, tag="ssq")
                for c in range(4):
                    nc.tensor.matmul(ssq, hssq_sb[:, c, :], qsqs[c],
                                     start=(c == 0), stop=(c == 3))
                lns = rsp.tile([8, 512], F32, tag="lns")
                nc.scalar.activation(out=lns, in_=ssq,
                                     func=mybir.ActivationFunctionType.Ln,
                                     bias=eps8)
                rs = rsp.tile([8, 512], BF16, tag="rs")
                nc.scalar.activation(out=rs, in_=lns,
                                     func=mybir.ActivationFunctionType.Exp,
                                     scale=-0.5)
                return t1, rs

            def rope_part(t1, rs, dst, dst_off, c_tab, s_tab, tab_off):
                """Apply inverse-rms scale + rotate-half RoPE, write dst."""
                t1s = t1sp.tile([P, N], BF16, tag="t1s")
                for c in range(4):
                    cs = slice(c * 512, (c + 1) * 512)
                    rsb = rsb_ps.tile([P, 512], F32)
                    nc.tensor.matmul(rsb, hbsel_sb[:, c, :], rs,
                                     start=True, stop=True)
                    rsbe = rsbep.tile([P, 512], BF16, tag="rsbe")
                    nc.scalar.activation(
                        out=rsbe, in_=rsb,
                        func=mybir.ActivationFunctionType.Copy)
                    nc.vector.tensor_mul(t1s[:, cs], t1[:, cs], rsbe)
                    # rotate-half across partitions via SBUF->SBUF DMA
                    t1r = t1rp.tile([P, 512], BF16, tag="t1r")
                    for h in range(HL):
                        b = h * DH
                        nc.sync.dma_start(out=t1r[b:b + 32, :],
                                          in_=t1s[b + 32:b + 64, cs])
                        nc.sync.dma_start(out=t1r[b + 32:b + 64, :],
                                          in_=t1s[b:b + 32, cs])
                    tab = slice(tab_off + c * 512, tab_off + (c + 1) * 512)
                    m1 = mrp.tile([P, 512], BF16, tag="m1")
                    nc.vector.tensor_mul(m1, t1s[:, cs], c_tab[:, tab])
                    r1 = mrp.tile([P, 512], BF16, tag="r1")
                    nc.vector.tensor_mul(r1, t1r, s_tab[:, tab])
                    sl = slice(dst_off + c * 512, dst_off + (c + 1) * 512)
                    nc.vector.tensor_add(dst[:, sl], m1, r1)

            def v_blockc(w_sb, src_sb, tbase, c):
                """V/CV projection chunk: vT via 512-wide passes, then XBAR
                transposes into the natural layout."""
                if not USE_XBAR_V:
                    for u in range(4):
                        t = tbase + 4 * c + u
                        tt = 4 * c + u
                        ps = proj_ps.tile([P, DL], F32)
                        for f in range(8):
                            nc.tensor.matmul(
                                ps, src_sb[:, f, tt * P:(tt + 1) * P],
                                w_sb[:, f, :], start=(f == 0), stop=(f == 7))
                        nc.scalar.activation(
                            out=v_nat[:, t, 0:64], in_=ps[:, 0:64],
                            func=mybir.ActivationFunctionType.Copy)
                        nc.scalar.activation(
                            out=v_nat[:, t, 65:129], in_=ps[:, 64:128],
                            func=mybir.ActivationFunctionType.Copy)
                    return
                ps = proj_ps.tile([P, 512], F32)
                for f in range(8):
                    nc.tensor.matmul(ps, w_sb[:, f, :],
                                     src_sb[:, f, c * 512:(c + 1) * 512],
                                     start=(f == 0), stop=(f == 7))
                vt = vtp.tile([P, 512], BF16, tag="vt")
                nc.scalar.activation(
                    out=vt, in_=ps, func=mybir.ActivationFunctionType.Copy)
                for u in range(4):
                    t = tbase + 4 * c + u
                    usl = slice(u * P, (u + 1) * P)
                    nc.sync.dma_start(out=v_nat[:, t, 0:64],
                                      in_=vt[0:64, usl], transpose=True)
                    nc.sync.dma_start(out=v_nat[:, t, 65:129],
                                      in_=vt[64:128, usl], transpose=True)

            t1q, rsq = proj_part(wq_sb, xT_sb)
            t1k, rsk = proj_part(wk_sb, xT_sb)
            rope_part(t1q, rsq, qTn, 0, cq_sb, sq_sb, 0)
            for c in range(4):
                v_blockc(wv_sb, xT_sb, 0, c)
            rope_part(t1k, rsk, kTn, 0, ckc_sb, cks_sb, 0)
            t1c, rsc_ = proj_part(wck_sb, yT_sb)
            for c in range(4):
                v_blockc(wcv_sb, yT_sb, 16, c)
            rope_part(t1c, rsc_, kTn, N, ckc_sb, cks_sb, N)

        # ---------------- phase 2: attention ----------------
        with tc.tile_pool(name="st_ps", bufs=2, space="PSUM") as st_ps, \
                tc.tile_pool(name="pv_ps", bufs=1, space="PSUM") as pv_ps, \
                tc.tile_pool(name="esp", bufs=2) as esp, \
                tc.tile_pool(name="esp2", bufs=2) as esp2, \
                tc.tile_pool(name="esbp", bufs=2) as esbp, \
                tc.tile_pool(name="yip", bufs=2) as yip, \
                tc.tile_pool(name="mip", bufs=2) as mip, \
                tc.tile_pool(name="uup", bufs=2) as uup, \
                tc.tile_pool(name="vvp", bufs=2) as vvp, \
                tc.tile_pool(name="ddp", bufs=2) as ddp:
            for qh in range(2):
                qsl = slice(qh * 1024, (qh + 1) * 1024)
                pv = [pv_ps.tile([65, 1024], F32, name=f"pv{h}", tag=f"pv{h}")
                      for h in range(HL)]
                for kc in range(NK // P):
                    # --- head 0: full ACT exp ---
                    st0 = st_ps.tile([P, 1024], F32, name="st0", tag="st")
                    for c in range(2):
                        nc.tensor.matmul(
                            st0[:, c * 512:(c + 1) * 512],
                            kTn[0:DH, kc * P:(kc + 1) * P],
                            qTn[0:DH, qh * 1024 + c * 512:
                                qh * 1024 + (c + 1) * 512],
                            start=True, stop=True)
                    e0 = esp.tile([P, 1024], BF16, name="es0", tag="es0")
                    nc.scalar.activation(out=e0, in_=st0,
                                         func=mybir.ActivationFunctionType.Exp,
                                         scale=SCALE)
                    # --- head 1: ACT exp on 768 cols, DVE exp2 on 256 ---
                    st1 = st_ps.tile([P, 1024], F32, name="st1", tag="st")
                    for c in range(2):
                        nc.tensor.matmul(
                            st1[:, c * 512:(c + 1) * 512],
                            kTn[DH:DL, kc * P:(kc + 1) * P],
                            qTn[DH:DL, qh * 1024 + c * 512:
                                qh * 1024 + (c + 1) * 512],
                            start=True, stop=True)
                    nact1 = 768 if USE_DVE_EXP else 1024
                    e1a = esp2.tile([P, nact1], BF16, name="es1a", tag="es1a")
                    nc.scalar.activation(out=e1a, in_=st1[:, 0:nact1],
                                         func=mybir.ActivationFunctionType.Exp,
                                         scale=SCALE)
                    if USE_DVE_EXP:
                        yi = yip.tile([P, 256], I32, tag="yi")
                        nc.vector.tensor_scalar(yi, st1[:, 768:1024], EA, EB,
                                                mybir.AluOpType.mult,
                                                mybir.AluOpType.add)
                        mi = mip.tile([P, 256], I32, tag="mi")
                        nc.vector.tensor_scalar(mi, yi, MANT, ONEBITS,
                                                mybir.AluOpType.bitwise_and,
                                                mybir.AluOpType.bitwise_or)
                        uu = uup.tile([P, 256], F32, tag="uu")
                        nc.vector.tensor_scalar(uu, mi.bitcast(F32), C2, C1,
                                                mybir.AluOpType.mult,
                                                mybir.AluOpType.add)
                        vv = vvp.tile([P, 256], F32, tag="vv")
                        nc.vector.tensor_mul(vv, uu, mi.bitcast(F32))
                        e1b = esbp.tile([P, 256], BF16, tag="e1b")
                        nc.vector.scalar_tensor_tensor(
                            e1b, vv, C0, yi.bitcast(F32),
                            mybir.AluOpType.add, mybir.AluOpType.mult)
                    # --- PV accumulation ---
                    kc_first, kc_last = kc == 0, kc == NK // P - 1
                    for c in range(2):
                        nc.tensor.matmul(
                            pv[0][:, c * 512:(c + 1) * 512],
                            v_stat(kc, 0), e0[:, c * 512:(c + 1) * 512],
                            start=kc_first, stop=kc_last)
                    nc.tensor.matmul(pv[1][:, 0:512], v_stat(kc, 1),
                                     e1a[:, 0:512],
                                     start=kc_first, stop=kc_last)
                    if USE_DVE_EXP:
                        # cols 512:768 and 768:1024 share one 2KB PSUM zero
                        # region: exactly one start (first writer, kc==0) and
                        # one stop (last writer, kc==31) for the pair.
                        nc.tensor.matmul(pv[1][:, 512:768], v_stat(kc, 1),
                                         e1a[:, 512:768],
                                         start=kc_first, stop=False)
                        nc.tensor.matmul(pv[1][:, 768:1024], v_stat(kc, 1),
                                         e1b, start=False, stop=kc_last)
                    else:
                        nc.tensor.matmul(pv[1][:, 512:1024], v_stat(kc, 1),
                                         e1a[:, 512:1024],
                                         start=kc_first, stop=kc_last)
                # Evict unnormalized oT + denominators, ship via A2A.
                dds = []
                for h in range(HL):
                    nc.vector.tensor_copy(oT[h * DH:(h + 1) * DH, qsl],
                                          pv[h][0:64, :])
                    dd = ddp.tile([1, 1024], F32, tag="dd", name=f"dd{h}")
                    nc.scalar.activation(
                        out=dd, in_=pv[h][64:65, :],
                        func=mybir.ActivationFunctionType.Copy)
                    dds.append(dd)
                # Batched A2A staging: one strided DMA for all 8 oT blocks
                # (iteration order p, j, q matches the SBUF source), one per
                # head for the denominator rows.
                BLK = NROW * P
                qb = a2a_in[qh, 0, 0:1, :]
                o_dst = bass.AP(tensor=qb.tensor, offset=qb.offset,
                                ap=[[P, P], [BLK, NCORES], [1, P]])
                nc.sync.dma_start(out=o_dst, in_=oT[:, qsl])
                for h in range(HL):
                    db = a2a_in[qh, 0, 128 + 2 * h:129 + 2 * h, :]
                    d_dst = bass.AP(tensor=db.tensor, offset=db.offset,
                                    ap=[[BLK, NCORES], [P, 2], [1, P]])
                    nc.sync.dma_start(out=d_dst,
                                      in_=dds[h][0:1, :].bitcast(BF16))
                nc.gpsimd.collective_compute(
                    "AllToAll", mybir.AluOpType.bypass,
                    replica_groups=[list(range(NCORES))],
                    ins=[a2a_in[qh]],
                    outs=[a2a_out[qh]],
                )

        # ---------------- phase 3: normalize + output projection ---------
        with tc.tile_pool(name="p3", bufs=1) as p3, \
                tc.tile_pool(name="djp", bufs=2) as djp, \
                tc.tile_pool(name="rdb_ps", bufs=2, space="PSUM") as rdb_ps, \
                tc.tile_pool(name="z_ps", bufs=2, space="PSUM") as z_ps, \
                tc.tile_pool(name="zout", bufs=2) as zout:
            for qh in range(2):
                of_sb = p3.tile([P, NCORES, P], BF16, name="of_sb",
                                tag=f"of{qh}")
                dj = djp.tile([16, P], F32, tag="dj")
                BLK = NROW * P
                qb = a2a_out[qh, 0, 0:1, :]
                o_src = bass.AP(tensor=qb.tensor, offset=qb.offset,
                                ap=[[P, P], [BLK, NCORES], [1, P]])
                nc.sync.dma_start(out=of_sb, in_=o_src)
                db = a2a_out[qh, 0, 128:132, :].bitcast(F32)
                d_src = bass.AP(tensor=db.tensor, offset=db.offset,
                                ap=[[BLK // 2, NCORES], [1, 2 * P]])
                nc.sync.dma_start(out=dj, in_=d_src)
                rdj = djp.tile([16, P], F32, tag="rdj")
                nc.vector.reciprocal(rdj, dj)
                rdjb = djp.tile([16, P], BF16, tag="rdjb")
                nc.vector.tensor_copy(rdjb, rdj)
                ofn_sb = p3.tile([P, NCORES, P], BF16, name="ofn_sb",
                                 tag=f"ofn{qh}")
                for i in range(NCORES):
                    rdb = rdb_ps.tile([P, P], F32)
                    nc.tensor.matmul(rdb, hdsel_sb[:, i, :], rdjb,
                                     start=True, stop=True)
                    nc.vector.tensor_mul(ofn_sb[:, i, :], of_sb[:, i, :], rdb)
                for nn in range(2):  # 2 output col chunks of 512
                    zp = z_ps.tile([P, 512], F32)
                    for i in range(NCORES):
                        nc.tensor.matmul(zp, ofn_sb[:, i, :],
                                         wo_sb[:, i, nn * 512:(nn + 1) * 512],
                                         start=(i == 0), stop=False)
                    nc.tensor.matmul(zp, ones1,
                                     bo_sb[:, nn * 512:(nn + 1) * 512],
                                     start=False, stop=True)
                    zs = zout.tile([P, 512], F32)
                    nc.scalar.activation(
                        out=zs, in_=zp,
                        func=mybir.ActivationFunctionType.Copy)
                    nc.sync.dma_start(out=out_ext[qh * P:(qh + 1) * P,
                                                  nn * 512:(nn + 1) * 512],
                                      in_=zs)
    return nc


def _bf16(a):
    return np.ascontiguousarray(a).astype(ml_dtypes.bfloat16)


def _rope_tables(npos, pos0, g_first, g_second, n_first):
    """Tables [128, npos] for transposed-layout rope with g folded in.

    Row j (within a head, duplicated for 2 local heads):
      out[j] = t[j]*C[j] + t[sigma(j)]*S[j]
      j <  32: C[j]=g[j]*cos[n,j],     S[j]=-g[j+32]*sin[n,j]
      j >= 32: C[j]=g[j]*cos[n,j-32],  S[j]=+g[j-32]*sin[n,j-32]
    g switches from g_first to g_second at position n_first.
    """
    inv = 1.0 / (ROPE_BASE ** (np.arange(0, DH, 2, dtype=np.float64) / DH))
    pos = np.arange(pos0, pos0 + npos, dtype=np.float64)
    ang = pos[:, None] * inv[None, :]          # [npos, 32]
    cos = np.cos(ang).T                         # [32, npos]
    sin = np.sin(ang).T
    C = np.zeros((DH, npos), np.float64)
    S = np.zeros((DH, npos), np.float64)
    g = np.zeros((DH, npos), np.float64)
    g[:, :n_first] = np.asarray(g_first, np.float64)[:, None]
    if n_first < npos:
        g[:, n_first:] = np.asarray(g_second, np.float64)[:, None]
    C[:32] = cos
    C[32:] = cos
    C *= g
    S[:32] = -sin
    S[32:] = sin
    Srot = np.concatenate([g[32:], g[:32]], axis=0)  # g[sigma(j)]
    S *= Srot
    C2_ = np.concatenate([C, C], axis=0)  # duplicate for 2 local heads
    S2_ = np.concatenate([S, S], axis=0)
    return _bf16(C2_), _bf16(S2_)


_NC_CACHE = None


def kernel(x, y, W_qkv, W_ckv, W_out, b_out, g_q, g_k, g_ck, n_heads):
    global LAST_RESULT, _NC_CACHE
    x = np.asarray(x, np.float32)
    y = np.asarray(y, np.float32)
    W_qkv = np.asarray(W_qkv, np.float32)
    W_ckv = np.asarray(W_ckv, np.float32)
    W_out = np.asarray(W_out, np.float32)
    b_out = np.asarray(b_out, np.float32)

    xT = _bf16(x[0].T)                       # [1024, 2048]
    yT = _bf16(y[0].T)
    Wq, Wk, Wv = (W_qkv[:, i * D:(i + 1) * D] for i in range(3))
    Wck, Wcv = (W_ckv[:, i * D:(i + 1) * D] for i in range(2))
    woh = _bf16(W_out)
    boh = _bf16(b_out[None, :])

    cqh, sqh = _rope_tables(N, 0, g_q, g_q, N)
    ckch, cksh = _rope_tables(NK, 0, g_k, g_ck, N)

    hssq = np.zeros((P, 4, 8), np.float32)
    hbsel = np.zeros((8, 4, P), np.float32)
    for c in range(4):
        for h in range(HL):
            hssq[h * DH:(h + 1) * DH, c, 2 * c + h] = 1.0 / DH
            hbsel[2 * c + h, c, h * DH:(h + 1) * DH] = 1.0
    hdsel = np.zeros((16, 8, P), np.float32)
    for i in range(NCORES):
        for h in range(HL):
            hdsel[2 * i + h, i, h * DH:(h + 1) * DH] = 1.0

    in_maps = []
    for c in range(NCORES):
        sl = slice(c * DL, (c + 1) * DL)
        in_maps.append({
            "xT": xT, "yT": yT,
            "wq": _bf16(Wq[:, sl]), "wk": _bf16(Wk[:, sl]),
            "wv": _bf16(Wv[:, sl]), "wck": _bf16(Wck[:, sl]),
            "wcv": _bf16(Wcv[:, sl]),
            "wo": woh, "bo": boh,
            "cq": cqh, "sq": sqh, "ckc": ckch, "cks": cksh,
            "hssq": _bf16(hssq.reshape(P, 32)),
            "hbsel": _bf16(hbsel.reshape(8, 512)),
            "hdsel": _bf16(hdsel.reshape(16, 8 * P)),
        })

    if _NC_CACHE is None:
        _NC_CACHE = build_nc()
        if not _NC_CACHE.is_finalized():
            _NC_CACHE.finalize()
    nc = _NC_CACHE

    res = run_bass_kernel_spmd(
        nc, in_maps, core_ids=list(range(NCORES)),
        trace=bool(os.environ.get("BASS_TRACE")),
    )
    LAST_RESULT = res
    out = np.empty((N, D), np.float32)
    for c in range(NCORES):
        o = np.asarray(res.results[c]["out"], np.float32)
        out[c * P:(c + 1) * P] = o[0:P]
        out[N // 2 + c * P:N // 2 + (c + 1) * P] = o[P:2 * P]
    return out[None, :, :]
